# revision 10
# baseline (speedup 1.0000x reference)
"""Causal self-attention kernel for 8 TRN2 NeuronCores.

Sharding: data-parallel over batch (B=8 -> 1 batch element per core).
Each core computes full 16-head causal attention for its batch element.
All matmuls run in bf16 with fp32 PSUM accumulation (~5.5e-3 rel err).

Per-core dataflow (L=1024, E=1024, H=16, D=64):
  XT  = x^T            host-pre-transposed bf16, loaded in ct-chunks
  V   = (x Wv + bv)|1  ct-outer accumulation into 8 concurrent PSUM
                       banks so the PE starts as soon as the first
                       XT/Wv chunks land (instead of after the full 4MB)
  QT  = Wq^T x^T + bq  [e, l] layout (stationary Wq blocks, moving XT)
  KT  = Wk^T x^T + bk  [e, l] layout
  attention            per head PAIR (2et, 2et+1), kt-major with
                       512-col score chunks; AV matmuls lag one chunk
                       behind their exp so ScalarE latency never stalls
                       the PE FIFO; ones-column in V emits softmax
                       denominators for free
  Y   = Yu[0:64]/s     s broadcast via DRAM round-trip + SWDGE bcast +
                       approx reciprocal (last pair: PE ones-matmul
                       broadcast instead, to kill the tail latency)
  out = Y^T.T Wo + bo  contraction over e-tiles; l-tiles 0..2 are woven
                       into the last head pair's kt loop to fill its
                       exp-latency gaps; stores in bf16 on alternating
                       HWDGE rings (host casts back to fp32)

DMA ring usage: sync ring = XT chunks, biases, half the out stores;
scalar ring = all weights (wv, wq/wk, wo) and the other half of the
stores; SWDGE (gpsimd) = bias/denominator broadcasts.
"""

import os
import sys

sys.path.insert(0, "/opt/trn_rl_repo")

import numpy as np

import concourse.bass as bass
import concourse.mybir as mybir
import concourse.tile as tile
from concourse import bacc
from concourse.bass_utils import run_bass_kernel_spmd
f32 = mybir.dt.float32
bf16 = mybir.dt.bfloat16
AF = mybir.ActivationFunctionType
OP = mybir.AluOpType

L = 1024
E = 1024
H = 16
D = 64
P = 128
NT = L // P  # 8 tiles along any 1024 dim
SCALE = 1.0 / np.sqrt(D)


def _build():
    nc = bacc.Bacc("TRN2", target_bir_lowering=False, debug=False, num_devices=8)
    wq = nc.dram_tensor("wq", [NT, P, NT, P], bf16, kind="ExternalInput").ap()
    wk = nc.dram_tensor("wk", [NT, P, NT, P], bf16, kind="ExternalInput").ap()
    wv = nc.dram_tensor("wv", [P, NT, E], bf16, kind="ExternalInput").ap()
    wo = nc.dram_tensor("wo", [P, NT, E], bf16, kind="ExternalInput").ap()
    bq = nc.dram_tensor("bq", [E], f32, kind="ExternalInput").ap()
    bk = nc.dram_tensor("bk", [E], f32, kind="ExternalInput").ap()
    bv = nc.dram_tensor("bv", [E], f32, kind="ExternalInput").ap()
    bo = nc.dram_tensor("bo", [E], f32, kind="ExternalInput").ap()
    xt_d = nc.dram_tensor("xt", [P, NT, L], bf16, kind="ExternalInput").ap()
    mask_d = nc.dram_tensor("mask01", [P, P], bf16, kind="ExternalInput").ap()
    out = nc.dram_tensor("out", [L, E], bf16, kind="ExternalOutput").ap()
    s_dram = nc.dram_tensor("s_scratch", [H, L], f32, kind="Internal").ap()

    with tile.TileContext(nc) as tc:
        _body(nc, tc, wq, wk, wv, wo, bq, bk, bv, bo, out, s_dram,
              xt_d, mask_d)
    return nc


def _chunks(kt):
    """(qc, lo, n) score chunks for k-tile kt, 512-col aligned, causal."""
    out = []
    for qc in range(kt // 4, 2):
        lo = max(qc * 512, kt * P)
        n = (qc + 1) * 512 - lo
        out.append((qc, lo, n))
    return out


def _body(nc, tc, wq, wk, wv, wo, bq, bk, bv, bo, out, s_dram, xt_d, mask_d):
    from contextlib import ExitStack

    ctx = ExitStack()
    with ctx:
        consts = ctx.enter_context(tc.tile_pool(name="consts", bufs=1))
        xt_pool = ctx.enter_context(tc.tile_pool(name="xt_pool", bufs=1))
        qt_pool = ctx.enter_context(tc.tile_pool(name="qt_pool", bufs=1))
        kt_pool = ctx.enter_context(tc.tile_pool(name="kt_pool", bufs=1))
        v_pool = ctx.enter_context(tc.tile_pool(name="v_pool", bufs=1))
        y_pool = ctx.enter_context(tc.tile_pool(name="y_pool", bufs=1))
        sst_pool = ctx.enter_context(tc.tile_pool(name="sst_pool", bufs=4))
        wblk_pool = ctx.enter_context(tc.tile_pool(name="wblk_pool", bufs=4))
        pt_pool = ctx.enter_context(tc.tile_pool(name="pt_pool", bufs=6))
        osb_pool = ctx.enter_context(tc.tile_pool(name="osb_pool", bufs=3))
        r_pool = ctx.enter_context(tc.tile_pool(name="r_pool", bufs=1))
        rh_pool = ctx.enter_context(tc.tile_pool(name="rh_pool", bufs=3))
        wo_pool = ctx.enter_context(tc.tile_pool(name="wo_pool", bufs=1))

        mask01 = consts.tile([P, P], bf16)
        nc.sync.dma_start(out=mask01, in_=mask_d)
        ones_t = consts.tile([D + 1, P], bf16)
        nc.vector.memset(ones_t, 0.0)
        nc.vector.memset(ones_t[D : D + 1, :], 1.0)
        bq_sb = consts.tile([P, NT], f32)
        nc.sync.dma_start(out=bq_sb, in_=bq.rearrange("(et p) -> p et", p=P))
        bk_sb = consts.tile([P, NT], f32)
        nc.sync.dma_start(out=bk_sb, in_=bk.rearrange("(et p) -> p et", p=P))
        bv_bc = consts.tile([P, E], f32)
        nc.gpsimd.dma_start(
            out=bv_bc,
            in_=bass.AP(tensor=bv.tensor, offset=bv.offset, ap=[[0, P], [1, E]]),
        )
        bo_bc = consts.tile([P, E], f32)  # DMA deferred into the et loop

        XT = xt_pool.tile([P, NT, L], bf16)  # [p, ct, l] = x^T[ct*128+p, l]
        QT = qt_pool.tile([P, NT, L], bf16)  # [p, et, l] = Q^T[et*128+p, l]
        KT = kt_pool.tile([P, NT, L], bf16)
        V = v_pool.tile([P, NT, H, D + 1], bf16)  # [p(l), lt, h, d | ones]
        Y = y_pool.tile([P, NT, L], bf16)  # [p, et, l] = y^T[et*128+p, l]
        Ybc = Y
        R = r_pool.tile([P, NT, L], f32)
        wo_r = wo_pool.tile([P, NT, E], bf16)

        nc.vector.memset(V[:, :, :, D : D + 1], 1.0)

        # ---- Phase 1+2a: chunked XT/Wv loads + ct-outer V projection ----
        # XT chunks on the sync HWDGE ring, wv chunks on the scalar ring:
        # both flow concurrently; chunk ct of each arrives ~together, and
        # the V matmuls for ct start as soon as it lands.
        with tc.tile_pool(name="wv_pool", bufs=1) as wvp, \
             tc.tile_pool(name="vps", bufs=1, space="PSUM") as vps:
            wv_sb = wvp.tile([P, NT, E], bf16)
            for ct in range(NT):
                nc.sync.dma_start(out=XT[:, ct, :], in_=xt_d[:, ct, :])
                nc.scalar.dma_start(out=wv_sb[:, ct, :], in_=wv[:, ct, :])
            for ec in range(2):
                psv = [
                    vps.tile([P, 512], f32, tag=f"v{lt}", name=f"psv{lt}")
                    for lt in range(NT)
                ]
                for ct in range(NT):
                    for lt in range(NT):
                        nc.tensor.matmul(
                            psv[lt],
                            XT[:, ct, lt * P : (lt + 1) * P],
                            wv_sb[:, ct, ec * 512 : (ec + 1) * 512],
                            start=(ct == 0),
                            stop=(ct == NT - 1),
                        )
                for lt in range(NT):
                    nc.vector.tensor_tensor(
                        out=V[:, lt, ec * 8 : (ec + 1) * 8, 0:D],
                        in0=psv[lt].rearrange("p (h d) -> p h d", h=8),
                        in1=bv_bc[:, ec * 512 : (ec + 1) * 512].rearrange(
                            "p (h d) -> p h d", h=8
                        ),
                        op=OP.add,
                    )

        # PSUM pools for the rest of the kernel (after vps is released):
        # pp (2 banks): proj / out-proj / ones-bcast, st (2): score chunks,
        # yh+yh2 (2+2): the pair's AV accumulators.
        pp = ctx.enter_context(tc.tile_pool(name="pp", bufs=2, space="PSUM"))
        sp = ctx.enter_context(tc.tile_pool(name="sp", bufs=2, space="PSUM"))
        yph = ctx.enter_context(tc.tile_pool(name="yph", bufs=1, space="PSUM"))
        yph2 = ctx.enter_context(tc.tile_pool(name="yph2", bufs=1, space="PSUM"))

        # ---- Phase 2b+3: QT/KT per et, then attention for the head pair ----
        for et in range(NT):
            if et == 1:
                # prefetch wo early on the scalar ring so the out-proj
                # (and its interleave into the last pair) never waits
                nc.scalar.dma_start(out=wo_r, in_=wo)
                nc.gpsimd.dma_start(
                    out=bo_bc,
                    in_=bass.AP(
                        tensor=bo.tensor, offset=bo.offset, ap=[[0, P], [1, E]]
                    ),
                )
            for (w_dram, b_sb, dst) in ((wq, bq_sb, QT), (wk, bk_sb, KT)):
                wqk_blk = wblk_pool.tile(
                    [P, NT, P], bf16, tag="wqkblk", name="wqk_blk"
                )
                nc.scalar.dma_start(out=wqk_blk, in_=w_dram[et])
                for lc in range(2):
                    ps = pp.tile([P, 512], f32, tag="pp")
                    for ct in range(NT):
                        nc.tensor.matmul(
                            ps,
                            wqk_blk[:, ct, :],
                            XT[:, ct, lc * 512 : (lc + 1) * 512],
                            start=(ct == 0),
                            stop=(ct == NT - 1),
                        )
                    nc.vector.tensor_scalar(
                        out=dst[:, et, lc * 512 : (lc + 1) * 512],
                        in0=ps,
                        scalar1=b_sb[:, et : et + 1],
                        scalar2=None,
                        op0=OP.add,
                    )

            last_pair = et == NT - 1
            _attention_pair(
                nc, tc, et, QT, KT, V, Ybc, s_dram, sp, (yph, yph2), pt_pool,
                sst_pool, mask01, last_pair, ones_t, rh_pool, pp,
                wo_r, bo_bc, osb_pool, out,
            )
            if not last_pair:
                for half in range(2):
                    hh = 2 * et + half
                    bsrc = bass.AP(
                        tensor=s_dram.tensor,
                        offset=s_dram[hh : hh + 1, :].offset,
                        ap=[[0, 64], [1, L]],
                    )
                    nc.gpsimd.dma_start(
                        out=R[half * 64 : (half + 1) * 64, et, :], in_=bsrc
                    )
                nc.vector.reciprocal_approx_fast(out=R[:, et, :], in_=R[:, et, :])
                for half in range(2):
                    rows = slice(half * 64, (half + 1) * 64)
                    nc.vector.tensor_tensor(
                        out=Ybc[rows, et, :],
                        in0=Y[rows, et, :],
                        in1=R[rows, et, :],
                        op=OP.mult,
                    )

        # ---- Phase 5 tail: out-proj for the l-tiles not yet emitted ----
        for lt in range(4, NT):
            _out_proj_lt(nc, lt, Ybc, wo_r, bo_bc, pp, osb_pool, out)


def _out_proj_lt(nc, lt, Ybc, wo_r, bo_bc, pp, osb_pool, out):
    for oc in range(2):
        ps = pp.tile([P, 512], f32, tag="pp", name="ps_out")
        for et in range(NT):
            nc.tensor.matmul(
                ps,
                Ybc[:, et, lt * P : (lt + 1) * P],
                wo_r[:, et, oc * 512 : (oc + 1) * 512],
                start=(et == 0),
                stop=(et == NT - 1),
            )
        osb = osb_pool.tile([P, 512], bf16)
        nc.vector.tensor_tensor(
            out=osb, in0=ps, in1=bo_bc[:, oc * 512 : (oc + 1) * 512], op=OP.add
        )
        eng = nc.sync if (lt + oc) % 2 == 0 else nc.scalar
        eng.dma_start(
            out=out[lt * P : (lt + 1) * P, oc * 512 : (oc + 1) * 512], in_=osb
        )


def _attention_pair(nc, tc, et, QT, KT, V, Ybc, s_dram, sp, yps, pt_pool,
                    sst_pool, mask01, last_pair, ones_t, rh_pool, pp,
                    wo_r, bo_bc, osb_pool, out):
    """Both heads (2et, 2et+1) of e-tile et, kt-major with 512-col chunks.

    Per chunk round the PE FIFO is [S_h, S_h', A_h(prev), A_h'(prev)]:
    the AV matmuls run one chunk behind their exp, so ScalarE's exp
    latency is hidden behind the next chunk's score matmuls. For the
    last pair, out-proj l-tiles are woven between kt rounds to fill
    the drained pipeline (no projection work is left to overlap).
    """
    heads = (2 * et, 2 * et + 1)
    yu = {}
    for hi in range(2):
        for qc in range(2):
            yu[(hi, qc)] = yps[hi].tile(
                [D + 1, 512], f32, tag=f"yu{qc}", name=f"yu{hi}{qc}"
            )

    chunk_list = []
    for kt in range(NT):
        for (qc, lo, n) in _chunks(kt):
            chunk_list.append((kt, qc, lo, n))

    pending = None  # (kt, qc, lo, n, pt_tiles)
    for (kt, qc, lo, n) in chunk_list:
        pts = []
        for hi, h in enumerate(heads):
            pb = (hi) * 64
            st = sp.tile([P, 512], f32, tag="st", name="st")
            nc.tensor.matmul(
                st[:, 0:n],
                KT[pb : pb + D, et, kt * P : (kt + 1) * P],
                QT[pb : pb + D, et, lo : lo + n],
                start=True,
                stop=True,
            )
            pt = pt_pool.tile([P, 512], bf16, tag="pt", name="pt")
            nc.scalar.activation(
                out=pt[:, 0:n], in_=st[:, 0:n], func=AF.Exp, scale=float(SCALE)
            )
            if lo == kt * P:
                # causal mask on the diagonal block: zero where q < k
                nc.vector.tensor_tensor(
                    out=pt[:, 0:P], in0=pt[:, 0:P], in1=mask01, op=OP.mult
                )
            pts.append(pt)
        if last_pair and qc == 1 and 4 <= kt <= 7:
            # weave an out-proj l-tile between the drained last pair's
            # score and AV matmuls (qc0 of both heads closed at kt=3)
            _out_proj_lt(nc, kt - 4, Ybc, wo_r, bo_bc, pp, osb_pool, out)
        if pending is not None:
            _emit_av(nc, et, heads, V, pending, yu, Ybc, s_dram, sst_pool,
                     last_pair, ones_t, rh_pool, pp)
        pending = (kt, qc, lo, n, pts)
    _emit_av(nc, et, heads, V, pending, yu, Ybc, s_dram, sst_pool,
             last_pair, ones_t, rh_pool, pp)


def _emit_av(nc, et, heads, V, chunk, yu, Ybc, s_dram, sst_pool, last_pair,
             ones_t, rh_pool, pp):
    kt, qc, lo, n, pts = chunk
    last_kt = min(NT - 1, (qc + 1) * 4 - 1)
    for hi, h in enumerate(heads):
        nc.tensor.matmul(
            yu[(hi, qc)][:, lo - qc * 512 : lo - qc * 512 + n],
            V[:, kt, h, :],
            pts[hi][:, 0:n],
            start=(kt == 0),
            stop=(kt == last_kt),
        )
        if kt == last_kt:
            _head_tail(
                nc, h, hi, qc, yu, Ybc, s_dram, sst_pool, last_pair, ones_t,
                rh_pool, pp,
            )


def _head_tail(nc, h, hi, qc, yu, Ybc, s_dram, sst_pool, last_pair, ones_t,
               rh_pool, pp):
    et = h // 2
    pb = hi * 64
    cols = slice(qc * 512, (qc + 1) * 512)
    yuq = yu[(hi, qc)]
    if last_pair:
        sstb = sst_pool.tile([D + 1, 512], bf16, tag="sstb", name="sstb")
        nc.vector.tensor_copy(out=sstb[D : D + 1, :], in_=yuq[D : D + 1, :])
        ps_bc = pp.tile([P, 512], f32, tag="pp", name="ps_bc")
        nc.tensor.matmul(
            ps_bc, ones_t[D : D + 1, :], sstb[D : D + 1, :],
            start=True, stop=True,
        )
        rh = rh_pool.tile([P, 512], f32, tag="rh", name="rh")
        nc.vector.reciprocal_approx_fast(out=rh, in_=ps_bc)
        nc.vector.tensor_tensor(
            out=Ybc[pb : pb + D, et, cols],
            in0=yuq[0:D, :],
            in1=rh[0:D, :],
            op=OP.mult,
        )
    else:
        sst = sst_pool.tile([D + 1, 512], f32, tag="sst", name="sst")
        nc.vector.tensor_copy(out=sst[D : D + 1, :], in_=yuq[D : D + 1, :])
        nc.sync.dma_start(out=s_dram[h : h + 1, cols], in_=sst[D : D + 1, :])
        nc.vector.tensor_copy(out=Ybc[pb : pb + D, et, cols], in_=yuq[0:D, :])


_COMPILED = None


def _get_compiled():
    global _COMPILED
    if _COMPILED is None:
        nc = _build()
        nc.compile()
        _COMPILED = nc
    return _COMPILED


def kernel(x, Wq, bq, Wk, bk, Wv, bv, Wo, bo, _trace=False):
    import ml_dtypes

    bfl = ml_dtypes.bfloat16
    nc = _get_compiled()
    x = np.ascontiguousarray(np.asarray(x, dtype=np.float32).astype(bfl))
    B = x.shape[0]
    assert B == 8 and x.shape[1] == L and x.shape[2] == E
    def _qk_layout(w):
        # [et, p, ct, e']: per-et contiguous [128, 8, 128] stationary blocks
        w = np.asarray(w, np.float32).astype(bfl)
        return np.ascontiguousarray(
            w.reshape(NT, P, NT, P).transpose(2, 1, 0, 3)
        )

    def _pct_layout(w):
        # [p, ct, e]: moving-operand blocks with contraction rows on partitions
        w = np.asarray(w, np.float32).astype(bfl)
        return np.ascontiguousarray(w.reshape(NT, P, E).transpose(1, 0, 2))

    common = {
        "wq": _qk_layout(Wq),
        "wk": _qk_layout(Wk),
        "wv": _pct_layout(Wv),
        "wo": _pct_layout(Wo),
        "bq": np.ascontiguousarray(np.asarray(bq, np.float32)),
        "bk": np.ascontiguousarray(np.asarray(bk, np.float32)),
        "bv": np.ascontiguousarray(np.asarray(bv, np.float32)),
        "bo": np.ascontiguousarray(np.asarray(bo, np.float32)),
    }
    common["mask01"] = np.tril(np.ones((P, P), np.float32)).T.astype(bfl)
    # xt[b]: [p, ct, l] with xt[b][p, ct, l] = x[b, l, ct*128+p]
    xt = np.ascontiguousarray(
        x.transpose(0, 2, 1).reshape(B, NT, P, L).transpose(0, 2, 1, 3)
    )
    in_maps = [dict(common, xt=xt[i]) for i in range(B)]
    res = run_bass_kernel_spmd(nc, in_maps, core_ids=list(range(8)), trace=_trace)
    outp = np.stack(
        [np.asarray(res.results[i]["out"]).astype(np.float32) for i in range(B)]
    )
    if _trace:
        kernel.last_exec_time_ns = res.exec_time_ns
    return outp
